# revision 1
# baseline (speedup 1.0000x reference)
"""Trainium2 Bass kernel for nn_AgeUGP_v2 (gnn_message_passing).

Reference pipeline:
  snp_h[b,n,f] = snp[b,n] * filters[f,n]
  gathered     = snp_h[:, snp_ids, :]
  per_gene     = segment_sum(gathered, node_seg)   # node_seg sorted
  sample_h     = per_gene.mean(-1)
  h1 = sample_h @ W1 ... tiny MLP tail

Algebraic collapse: the filter axis F is only averaged at the end, so
  sample_h[b,g] = sum_{i in seg g} snp[b, id_i] * fbar[id_i],
  fbar = mean(filters, axis=0).

Device strategy (8 NeuronCores, genes sharded across cores):
  - SNP axis padded to 64 chunks of 8192.  4 table phases; in phase T the
    128 partitions hold (chunk, batch) tables of v = snp * fbar in f32:
    partition p = 16g + 8h + b holds chunk 16T + g + 8h, batch b.
    Tables are built in-place from a host-permuted copy of snp (wide
    quarter DMAs); fbar is produced fused: a host-permuted bf16 copy of
    filters (rows on partitions) is hit with a single 1/8-valued
    mean+replicate PE matmul per 512 columns, whose PSUM output directly
    multiplies the table on DVE.
  - One merged pass per table: gpsimd ap_gather gathers the nodes of both
    chunk halves in one gene-ordered stream (group g's shared index stream
    is applied to all 16 lanes; each node is valid on its half's 8 lanes,
    junk elsewhere is excluded by the combine).  A DVE tensor_tensor_scan
    (fp32, in place over the gather buffer) forms prefix sums; a second
    ap_gather extracts prefixes at the A-end and B-end gene boundaries;
    one adjacent-difference gives per-(half,gene,batch) partials with no
    padding inflation and no masking.  (ap_gather index APs must start at
    a tile base: sliced index APs silently misread on HW.)
  - Per pass, PE matmuls against 0/1 lane-selection columns (selA for the
    A-half lanes, selB for B) form the valid-lane sums in PSUM; DVE
    accumulates into sample_h [gene, batch].
  - PE matmul with the core's W1 shard (bf16, host-permuted for wide
    loads) -> partial h1 [8, 1024].
  - host sums the 8 partials and runs the tiny MLP tail (0.01% of FLOPs).
Emission is software-pipelined (gather p+1 ahead of pass-p tail; tables
double-buffered, snp/filter loads interleaved at quarter granularity) so
Pool/DVE/DMA overlap at ~23.5us per table cycle each.
"""

import numpy as np

B = 8
N_SNPS = 500000
N_NODES = 2000000
N_GENES = 20000
N_FILT = 8
N_CORES = 8
BN_EPS = 1e-5

_P = 128
_NCHUNK = 64  # SNP chunks
_NTAB = 4  # table phases
_NPASS = 4  # gather passes (one per table; chunk halves merged)
_EPAD = 16


def make_cfg(n_snps, n_genes, n_cores, chunk, d1, J, qchunks):
    snp_pad = _NCHUNK * chunk
    piece = snp_pad // _P
    assert snp_pad >= n_snps
    assert J % 16 == 0
    gpc = n_genes // n_cores
    jt = -(-gpc // _P)
    gpad_ = jt * _P
    ns = gpad_ + gpc + 1  # boundaries: dummy + gpad A-ends + gpc B-ends
    nspad = -(-ns // _EPAD) * _EPAD
    return dict(
        n_snps=n_snps, snp_pad=snp_pad, chunk=chunk, piece=piece,
        n_genes=n_genes, n_cores=n_cores, gpc=gpc, gpad=jt * _P, jt=jt,
        d1=d1, J=J, qchunks=qchunks, ns=ns, nspad=nspad,
    )


def full_cfg(J):
    return make_cfg(N_SNPS, N_GENES, N_CORES, 8192, 1024, J, 16)


# ---------------------------------------------------------------- device program
def build_program(cfg):
    import concourse.bass as bass
    import concourse.bacc as bacc
    import concourse.mybir as mybir
    import concourse.tile as tile

    fp32 = mybir.dt.float32
    bf16 = mybir.dt.bfloat16
    i16 = mybir.dt.int16

    chunk, piece, snp_pad = cfg["chunk"], cfg["piece"], cfg["snp_pad"]
    jt, d1, J = cfg["jt"], cfg["d1"], cfg["J"]
    gpad, nspad, gpc = cfg["gpad"], cfg["nspad"], cfg["gpc"]

    nc = bacc.Bacc(
        "TRN2", target_bir_lowering=False, debug=False, num_devices=cfg["n_cores"]
    )

    tp = snp_pad // (_NTAB * _P)  # fbar T-slice columns per partition
    n_sp = chunk // tp  # routing matrices (shared across T)
    assert 16 * n_sp == _P
    qf = 4 if tp % 4 == 0 else 1  # fbar sub-loads per T-slice
    assert tp % qf == 0

    snp_in = nc.dram_tensor(
        "snp_perm", [_P, _NTAB * chunk], fp32, kind="ExternalInput"
    )
    filt_in = nc.dram_tensor(
        "filt_perm", [_P, _NTAB * chunk], bf16, kind="ExternalInput"
    )
    gidx_in = nc.dram_tensor(
        "gidx", [_P, _NPASS * (J // 16)], i16, kind="ExternalInput"
    )
    eidx_in = nc.dram_tensor(
        "eidx", [_P, _NPASS * (nspad // 16)], i16, kind="ExternalInput"
    )
    sel_in = nc.dram_tensor("sel", [_P, 16], bf16, kind="ExternalInput")
    route_in = nc.dram_tensor("mroute", [_P, _P], bf16, kind="ExternalInput")
    w1_in = nc.dram_tensor("w1c", [_P, jt * d1], bf16, kind="ExternalInput")
    h1_out = nc.dram_tensor("h1p", [B, d1], fp32, kind="ExternalOutput")

    with tile.TileContext(nc) as tc:
        with (
            tc.tile_pool(name="per", bufs=1) as perpool,
            tc.tile_pool(name="tab", bufs=2) as tabpool,
            tc.tile_pool(name="fbr", bufs=1) as fbrpool,
            tc.tile_pool(name="gs", bufs=2) as gspool,
            tc.tile_pool(name="ft", bufs=2) as ftpool,
            tc.tile_pool(name="ex", bufs=1) as expool,
            tc.tile_pool(name="dd", bufs=1) as ddpool,
            tc.tile_pool(name="w1", bufs=3) as w1pool,
            tc.tile_pool(name="ps", bufs=4, space="PSUM") as pspool,
            tc.tile_pool(name="psw", bufs=1, space="PSUM") as pswpool,
            tc.tile_pool(name="psh", bufs=2, space="PSUM") as pshpool,
        ):
            # mean+replication routing matrix
            route = perpool.tile([_P, _P], bf16, tag="route")
            nc.sync.dma_start(route[:], route_in.ap())
            sel = perpool.tile([_P, 16], bf16, tag="sel")
            nc.sync.dma_start(sel[:], sel_in.ap())
            zs = perpool.tile([_P, 1], fp32, tag="zs")
            nc.vector.memset(zs[:], 0.0)

            # SBUF accumulator for sample_h [gene-tile, (t, b)]
            sh = perpool.tile([_P, jt * B], fp32, tag="sh")
            nc.vector.memset(sh[:], 0.0)

            vtabs = {}
            rc = min(512, chunk)
            nblk = chunk // rc
            nhv = 4 if nblk % 4 == 0 else 1
            fhalf = chunk // nhv

            def emit_table(T):
                # filters T-slice (rows on partitions via host perm), cast to
                # bf16 per block; one mean+replicate matmul per 512 columns
                vtab = tabpool.tile([_P, chunk], fp32, tag="vtab", name=f"vtab{T}")
                for hv in range(nhv):
                    # interleave snp/filter quarter-loads so multiply blocks
                    # start as early as possible
                    nc.sync.dma_start(
                        vtab[:, hv * fhalf : (hv + 1) * fhalf],
                        snp_in.ap()[:, T * chunk + hv * fhalf :
                                    T * chunk + (hv + 1) * fhalf],
                    )
                    ft = ftpool.tile(
                        [_P, fhalf], bf16, tag="ftl", name=f"ftl{T}_{hv}"
                    )
                    nc.sync.dma_start(
                        ft[:],
                        filt_in.ap()[:, T * chunk + hv * fhalf :
                                     T * chunk + (hv + 1) * fhalf],
                    )
                    for blk in range(nblk // nhv):
                        pr = pspool.tile([_P, rc], fp32, tag="pr", name="pr")
                        nc.tensor.matmul(
                            pr[:], route[:], ft[:, blk * rc : (blk + 1) * rc],
                            start=True, stop=True,
                        )
                        ks = slice(hv * fhalf + blk * rc,
                                   hv * fhalf + (blk + 1) * rc)
                        nc.vector.tensor_mul(vtab[:, ks], vtab[:, ks], pr[:])
                vtabs[T] = vtab

            def emit_gather(pidx):
                gidx = gspool.tile(
                    [_P, J // 16], i16, tag="gidx", name=f"gidx{pidx}"
                )
                nc.sync.dma_start(
                    gidx[:],
                    gidx_in.ap()[:, pidx * (J // 16) : (pidx + 1) * (J // 16)],
                )
                gout = gspool.tile([_P, J], fp32, tag="gout", name=f"gout{pidx}")
                nc.gpsimd.ap_gather(
                    gout[:], vtabs[pidx][:], gidx[:],
                    channels=_P, num_elems=chunk, d=1, num_idxs=J,
                )
                return gout

            def emit_tail(pidx, gout):
                # in-place prefix scan: safe, the scan never reads its output
                q = gout
                zbc = bass.AP(zs.tensor, zs[:].offset, [zs[:].ap[0], [0, J]])
                nc.vector.tensor_tensor_scan(
                    q[:], zbc, gout[:], 0.0,
                    op0=mybir.AluOpType.add, op1=mybir.AluOpType.add,
                )
                eidx = gspool.tile(
                    [_P, nspad // 16], i16, tag="eidx", name=f"eidx{pidx}"
                )
                nc.sync.dma_start(
                    eidx[:],
                    eidx_in.ap()[:, pidx * (nspad // 16) : (pidx + 1) * (nspad // 16)],
                )
                ex = expool.tile([_P, nspad], fp32, tag="ex", name=f"ex{pidx}")
                nc.gpsimd.ap_gather(
                    ex[:], q[:], eidx[:],
                    channels=_P, num_elems=J, d=1, num_idxs=nspad,
                )
                # E = [Q0, A-ends (gpad, padded), B-ends (gpc)]; adjacent
                # diffs give ddA at [0,gpad) and ddB at [gpad, gpad+gpc)
                nd = gpad + gpc
                dd = ddpool.tile([_P, 2 * gpad], bf16, tag="dd", name=f"dd{pidx}")
                if 2 * gpad > nd:
                    nc.vector.memset(dd[:, nd:], 0.0)
                nc.vector.tensor_sub(dd[:, :nd], ex[:, 1 : nd + 1], ex[:, :nd])
                pst = pshpool.tile([_P, jt * B], fp32, tag="pst", name="pst")
                for t in range(jt):
                    nc.tensor.matmul(
                        pst[:, t * B : (t + 1) * B],
                        dd[:, t * _P : (t + 1) * _P],
                        sel[:, :8],
                        start=True, stop=False,
                    )
                    nc.tensor.matmul(
                        pst[:, t * B : (t + 1) * B],
                        dd[:, gpad + t * _P : gpad + (t + 1) * _P],
                        sel[:, 8:],
                        start=False, stop=True,
                    )
                nc.vector.tensor_add(sh[:], sh[:], pst[:])

            # software-pipelined emission: gather(p+1) ahead of tail(p)
            emit_table(0)
            gouts = {0: emit_gather(0)}
            for p in range(_NPASS):
                if p + 1 < _NTAB:
                    emit_table(p + 1)
                if p + 1 < _NPASS:
                    gouts[p + 1] = emit_gather(p + 1)
                emit_tail(p, gouts.pop(p))

            shb = perpool.tile([_P, jt * B], bf16, tag="shb")
            nc.vector.tensor_copy(shb[:], sh[:])

            # ---- W1 matmul: accumulate over jt K-tiles --------------------
            n_half = min(512, d1)
            n_banks = -(-d1 // n_half)
            pss = []
            for nb in range(n_banks):
                pst = pswpool.tile([_P, n_half], fp32, tag=f"ps{nb}", name=f"ps{nb}")
                pss.append(pst)
            wgrp = 5 if jt % 5 == 0 else 1  # K-tiles per W1 load
            for jg in range(jt // wgrp):
                w1t = w1pool.tile([_P, wgrp * d1], bf16, tag="w1t")
                nc.sync.dma_start(
                    w1t[:],
                    w1_in.ap()[:, jg * wgrp * d1 : (jg + 1) * wgrp * d1],
                )
                for jl in range(wgrp):
                    j = jg * wgrp + jl
                    lhsT = shb[:, j * B : (j + 1) * B]
                    for nb in range(n_banks):
                        nc.tensor.matmul(
                            pss[nb][:B, :],
                            lhsT,
                            w1t[:, jl * d1 + nb * n_half : jl * d1 + (nb + 1) * n_half],
                            start=(j == 0),
                            stop=(j == jt - 1),
                        )

            h1 = perpool.tile([B, d1], fp32, tag="h1")
            for nb in range(n_banks):
                nc.vector.tensor_copy(
                    h1[:, nb * n_half : (nb + 1) * n_half], pss[nb][:B, :]
                )
            nc.sync.dma_start(h1_out.ap(), h1[:])

    nc.compile()
    return nc


# ---------------------------------------------------------------- host side
def _wrap16(streams):
    """[8, J] per-group streams -> [128, J//16] wrapped-16 layout."""
    ngrp, J = streams.shape
    assert ngrp == 8 and J % 16 == 0
    out = np.zeros((_P, J // 16), streams.dtype)
    for g in range(8):
        out[g * 16 : (g + 1) * 16, :] = streams[g].reshape(J // 16, 16).T
    return out


def prep_inputs(cfg, snp, snp_ids, node_seg, filters, W1):
    """Index/metadata preprocessing + zero-padding + pure layout permutation;
    all value computation happens on device."""
    import ml_dtypes

    snp_pad_n, chunk, piece = cfg["snp_pad"], cfg["chunk"], cfg["piece"]
    gpc, gpad, d1 = cfg["gpc"], cfg["gpad"], cfg["d1"]
    n_genes, n_snps = cfg["n_genes"], cfg["n_snps"]
    J, nspad = cfg["J"], cfg["nspad"]
    n_cores = cfg["n_cores"]
    ppc = chunk // piece

    snp_p = np.zeros((B, snp_pad_n), np.float32)
    snp_p[:, :n_snps] = np.asarray(snp, np.float32)
    filt_p = np.zeros((B, snp_pad_n), np.float32)
    filt_p[:, :n_snps] = np.asarray(filters, np.float32)
    # filt_perm[q, T*chunk + k] = filters[q%8, (16T + q//8)*chunk + k]
    filt_perm = np.empty((_P, _NTAB * chunk), np.float32)  # cast below
    for T in range(_NTAB):
        view = filt_p[:, 16 * T * chunk : (16 * T + 16) * chunk].reshape(
            B, 16, chunk
        )  # [r, sp, k]
        filt_perm[:, T * chunk : (T + 1) * chunk] = (
            view.transpose(1, 0, 2).reshape(_P, chunk)
        )
    filt_perm_bf = filt_perm.astype(ml_dtypes.bfloat16)

    # pure layout permutation: row 16g+8h+b, cols [T*chunk,(T+1)*chunk) holds
    # snp[b, (16T+g+8h)*chunk : +chunk]
    snp_perm = np.empty((_P, _NTAB * chunk), np.float32)
    for T in range(_NTAB):
        view = snp_p[:, 16 * T * chunk : (16 * T + 16) * chunk].reshape(
            B, 2, 8, chunk
        )  # [b, h, g, k]
        snp_perm[:, T * chunk : (T + 1) * chunk] = (
            view.transpose(2, 1, 0, 3).reshape(_P, chunk)
        )

    # mean+replicate routing: out[m, j] = (1/8) sum_r filters[r, c(m)*chunk+j]
    # lhsT[q, m] = 1/8 iff q//8 == g(m) + 8*h(m)
    mroute = np.zeros((_P, _P), ml_dtypes.bfloat16)
    m = np.arange(_P)
    g, hb = m // 16, m % 16
    hh = hb // 8
    mroute[:, :] = 0
    for mm in range(_P):
        spt = g[mm] + 8 * hh[mm]
        mroute[spt * 8 : spt * 8 + 8, mm] = 1.0 / N_FILT

    sel = np.zeros((_P, 16), ml_dtypes.bfloat16)
    for p in range(_P):
        sel[p, p % 16] = 1.0

    ids = np.asarray(snp_ids).astype(np.int64)
    seg = np.asarray(node_seg).astype(np.int64)
    gene_starts = np.searchsorted(seg, np.arange(0, n_genes + 1))
    node_chunk = ids // chunk
    node_lidx = (ids % chunk).astype(np.int16)

    W1f = np.asarray(W1, np.float32)
    per_core = []
    for c in range(n_cores):
        lo, hi = gene_starts[c * gpc], gene_starts[(c + 1) * gpc]
        cid_chunk = node_chunk[lo:hi]
        cid_lidx = node_lidx[lo:hi]
        cid_gene = seg[lo:hi] - c * gpc  # local gene, sorted ascending

        gidx = np.zeros((_NPASS, 8, J), np.int16)
        eidx = np.zeros((_NPASS, 8, nspad), np.int16)
        for T in range(_NTAB):
            for g_ in range(8):
                chA, chB = 16 * T + g_, 16 * T + 8 + g_
                mA = cid_chunk == chA
                mB = cid_chunk == chB
                lidxA, lgeneA = cid_lidx[mA], cid_gene[mA]
                lidxB, lgeneB = cid_lidx[mB], cid_gene[mB]
                cntA, cntB = len(lidxA), len(lidxB)
                assert cntA + cntB + 1 <= J, f"bucket {cntA+cntB} exceeds J={J}"
                # merged stream: [dummy, chunk-A nodes by gene, chunk-B nodes]
                gidx[T, g_, 1 : 1 + cntA] = lidxA
                gidx[T, g_, 1 + cntA : 1 + cntA + cntB] = lidxB
                # boundary positions: [0, A-ends (gpad, pad=end-of-A), B-ends]
                FA = np.searchsorted(lgeneA, np.arange(1, gpc + 1))
                FB = cntA + np.searchsorted(lgeneB, np.arange(1, gpc + 1))
                pos = np.zeros(nspad, np.int64)
                pos[1 : 1 + gpc] = FA
                pos[1 + gpc : 1 + gpad] = FA[-1]
                pos[1 + gpad : 1 + gpad + gpc] = FB
                pos[1 + gpad + gpc :] = FB[-1]
                eidx[T, g_] = pos.astype(np.int16)

        w1c = np.zeros((gpad, d1), np.float32)
        w1c[:gpc] = W1f[c * gpc : (c + 1) * gpc]
        jt_ = gpad // _P
        # w1 perm: [p, j*d1 + col] = w1c[j*128 + p, col]
        w1perm = np.ascontiguousarray(
            w1c.reshape(jt_, _P, d1).transpose(1, 0, 2).reshape(_P, jt_ * d1)
        ).astype(ml_dtypes.bfloat16)
        gidx_all = np.concatenate(
            [_wrap16(gidx[p]) for p in range(_NPASS)], axis=1
        )
        eidx_all = np.concatenate(
            [_wrap16(eidx[p]) for p in range(_NPASS)], axis=1
        )
        core_map = dict(
            snp_perm=snp_perm, filt_perm=filt_perm_bf, sel=sel, w1c=w1perm,
            mroute=mroute, gidx=gidx_all, eidx=eidx_all,
        )
        per_core.append(core_map)
    return per_core


def host_tail(h1_sum, b1, g1, be1, W2, b2, g2, be2, W3, b3, g3, be3,
              Wh1, bh1, gh, beh, Wh2, bh2):
    def bn(x, g, be):
        return x * (g / np.sqrt(np.float32(1.0 + BN_EPS))) + be

    relu = lambda x: np.maximum(x, np.float32(0.0))
    h = relu(bn(h1_sum + b1, g1, be1))
    h = relu(bn(h @ W2 + b2, g2, be2))
    feat = relu(bn(h @ W3 + b3, g3, be3))
    m = relu(bn(feat[:, :15] @ Wh1 + bh1, gh, beh))
    return (m @ Wh2 + bh2).astype(np.float32)


def pick_J(snp_ids, node_seg, chunk=8192):
    ids = np.asarray(snp_ids).astype(np.int64)
    seg = np.asarray(node_seg).astype(np.int64)
    gpc = N_GENES // N_CORES
    gene_starts = np.searchsorted(seg, np.arange(0, N_GENES + 1, gpc))
    mx = 0
    for c in range(N_CORES):
        lo, hi = gene_starts[c], gene_starts[c + 1]
        cnt = np.bincount(ids[lo:hi] // chunk, minlength=_NCHUNK)
        comb = cnt.reshape(_NTAB, 2, 8).sum(axis=1)  # chunk + chunk+8 merged
        mx = max(mx, int(comb.max()))
    J = -(-(mx + 1) // 16) * 16
    # int16 stream/boundary indices: fail loudly rather than wrap silently
    assert J <= 32752, f"pass stream length {J} exceeds int16 index range"
    return J


_CACHE = {}


def kernel(snp, snp_ids, node_seg, filters, W1, b1, g1, be1, W2, b2, g2, be2,
           W3, b3, g3, be3, Wh1, bh1, gh, beh, Wh2, bh2):
    from concourse import bass_utils

    J = pick_J(snp_ids, node_seg)
    cfg = full_cfg(J)

    key = ("full", J)
    if key not in _CACHE:
        _CACHE[key] = build_program(cfg)
    nc = _CACHE[key]

    in_maps = prep_inputs(cfg, snp, snp_ids, node_seg, filters, W1)
    res = bass_utils.run_bass_kernel_spmd(
        nc, in_maps, core_ids=list(range(cfg["n_cores"]))
    )
    h1_sum = np.zeros((B, cfg["d1"]), np.float32)
    for c in range(cfg["n_cores"]):
        h1_sum += res.results[c]["h1p"]

    f32 = lambda x: np.asarray(x, np.float32)
    return host_tail(h1_sum, f32(b1), f32(g1), f32(be1), f32(W2), f32(b2),
                     f32(g2), f32(be2), f32(W3), f32(b3), f32(g3), f32(be3),
                     f32(Wh1), f32(bh1), f32(gh), f32(beh), f32(Wh2), f32(bh2))

